# revision 1
# baseline (speedup 1.0000x reference)
"""AttnBlock (GroupNorm + single-head self-attention + residual) on 8 trn2 cores.

Sharding: data-parallel over batch (4 samples) x query-halves (2) = 8 cores.
Each core gets its sample's tokens rotated so its 2048 queries are tokens
0:2048 of its input; GroupNorm stats and attention key-sums are invariant to
token order.

Weight reassociation (host precomputes weight-only products; all runtime
FLOPs stay on device): scores S = hn (Wq Wk^T) hn^T = x M2 x^T + per-key bias
(M2 = diag(a) M diag(a) absorbs the GroupNorm affine; per-query terms cancel
in softmax; the per-key term v = x . (SCALE a(.)(M^T d + Wk bq)) rides the
exp's per-partition bias). Values: attn hn Wv Wo = (attn x) diag(a)(Wv Wo)
+ const row. So K/V/Q/O projections collapse into ONE P = M2^T x projection
of the queries plus raw-x operands for keys and values -- no K/V projections,
no cross-core duplication, no collectives.

Device (per core):
  - GroupNorm: bn_stats per channel over tokens; per-chunk group combine /
    broadcast via tiny PE matmuls with 0/1 matrices -> a (scale), d (shift).
  - M2/W2~ built in place by row- and column-scaling host-shipped M, Wv Wo.
  - Attention per query block, software-pipelined by one k-chunk: S^T[k,q]
    accumulated over c chunks (lhsT = raw x^T); ACT exp (scale=C^-0.5,
    bias=v_k) PSUM->SBUF fp16; O^T accumulates over 32 k-chunks with
    lhsT = raw token-major x; sum-exp accumulates on DVE in fp32; softmax
    normalization deferred to the output projection (q = partition dim).
"""
import os
import numpy as np
import ml_dtypes
from contextlib import ExitStack

import concourse.bass as bass
import concourse.tile as tile
from concourse.tile_rust import add_dep_helper
from concourse import bacc, mybir
from concourse.bass_utils import run_bass_kernel_spmd

B, H, W, C = 4, 64, 64, 512
N = H * W            # 4096 tokens / sample
NQ = N // 2          # 2048 queries / core
G = 32
GS = C // G          # 16 channels / group
EPS = 1e-6
CH = C // 128        # 4 channel chunks
KC = N // 128        # 32 key chunks
TT = N // 512        # token tiles for projections
QB = NQ // 512       # query blocks
SCALE = float(C) ** -0.5

FP16 = os.environ.get("FP16", "1") == "1"
BF = mybir.dt.float16 if FP16 else mybir.dt.bfloat16
F32 = mybir.dt.float32
A = mybir.AluOpType
AF = mybir.ActivationFunctionType

_CACHE = {}
DEDUP_KV = os.environ.get("DEDUP_KV", "1") == "1"   # halve K/V proj work via pair AllGather


def _build(dedup=None):
    dedup = DEDUP_KV if dedup is None else dedup
    nc = bacc.Bacc("TRN2", target_bir_lowering=False, debug=False, num_devices=8)
    xtb = nc.dram_tensor("xtb", [C, N], BF, kind="ExternalInput").ap()
    xtok = nc.dram_tensor("xtok", [N, C], BF, kind="ExternalInput").ap()
    xres = nc.dram_tensor("xres", [NQ, C], F32, kind="ExternalInput").ap()
    wts = {k: nc.dram_tensor(k, [128, CH, C], BF, kind="ExternalInput").ap()
           for k in ("m", "mt", "w2")}
    pcs = {k: nc.dram_tensor(k, [128, CH], F32, kind="ExternalInput").ap()
           for k in ("gns", "gnb")}
    rows = {k: nc.dram_tensor(k, [1, C], F32, kind="ExternalInput").ap()
            for k in ("g", "h2")}
    gmat = nc.dram_tensor("gmat", [128, 8], F32, kind="ExternalInput").ap()
    gmat2 = nc.dram_tensor("gmat2", [8, 128], F32, kind="ExternalInput").ap()
    ones = nc.dram_tensor("ones", [128, 1], F32, kind="ExternalInput").ap()
    out = nc.dram_tensor("out", [NQ, C], F32, kind="ExternalOutput").ap()

    with tile.TileContext(nc) as tc, ExitStack() as ctx:
        pers = ctx.enter_context(tc.tile_pool(name="pers", bufs=1))
        dram = ctx.enter_context(tc.tile_pool(name="dram", bufs=2, space="DRAM"))

        # ---------- phase 1: load x^T first (DMA-bandwidth critical path) ----
        xT = pers.tile([128, CH, N], BF, name="xT")  # chunk j = channels 128j..128j+127
        xtb_r = xtb.rearrange("(a p) t -> p a t", p=128)
        for j in range(CH):
            for h in range(2):
                sl = slice(h * N // 2, (h + 1) * N // 2)
                nc.sync.dma_start(out=xT[:, j, sl], in_=xtb_r[:, j, sl])

        # ---------- constants / weights (after x in the DMA queues) ----------
        def load_pc(k):  # [128, CH] per-partition chunk columns
            t = pers.tile([128, CH], F32, name=f"{k}_pc")
            nc.sync.dma_start(out=t, in_=pcs[k])
            return t

        gns_pc, gnb_pc = load_pc("gns"), load_pc("gnb")
        gmat_sb = pers.tile([128, 8], F32, name="gmat_sb")
        nc.sync.dma_start(out=gmat_sb, in_=gmat)
        gmat2_sb = pers.tile([8, 128], F32, name="gmat2_sb")
        nc.sync.dma_start(out=gmat2_sb, in_=gmat2)
        ones_sb = pers.tile([128, 1], F32, name="ones_sb")
        nc.sync.dma_start(out=ones_sb, in_=ones)
        ident1 = pers.tile([1, 1], F32, name="ident1")
        nc.vector.memset(ident1, 1.0)
        brow = {}
        for k in ("g", "h2"):
            t = pers.tile([1, C], F32, name=f"{k}_row")
            nc.sync.dma_start(out=t, in_=rows[k])
            brow[k] = t
        w_sb = {}
        for k in ("m", "w2", "mt"):
            t = pers.tile([128, CH, C], BF, name=f"{k}_sb")
            nc.sync.dma_start(out=t, in_=wts[k])
            w_sb[k] = t
        # token-major raw x (the "values" after the W2 reassociation)
        xtok_sb = pers.tile([128, KC, C], BF, name="xtok_sb")
        nc.sync.dma_start(out=xtok_sb, in_=xtok.rearrange("(a p) c -> p a c", p=128))

        with tc.tile_pool(name="stats", bufs=2) as stp, \
             tc.tile_pool(name="stps", bufs=1, space="PSUM") as stps:
            # Fully per-chunk stats pipeline: chunk j's group stats, broadcast,
            # and weight-row scaling complete as soon as its bn_stats land, so
            # the j-th accumulation step of the first projections can start
            # while later chunks' stats are still being computed.
            #   mean = S1/16; var = (16*S2 - S1^2)/256; rstd = 16/sqrt(16*S2 - S1^2 + 256*eps)
            a_pc = stp.tile([128, CH], F32, tag="a_pc", bufs=1)
            d_pc = stp.tile([128, CH], F32, tag="d_pc", bufs=1)
            d_bf = stp.tile([128, CH], BF, tag="d_bf", bufs=1)
            ra_pc = stp.tile([128, CH], F32, tag="ra_pc", bufs=1)
            eps_t = stp.tile([8, 1], F32, tag="eps", bufs=1)
            nc.vector.memset(eps_t, float(GS * GS) * EPS)
            for j in range(CH):
                st = stp.tile([128, 8, 6], F32, tag="st")
                xv = xT[:, j, :].rearrange("p (s f) -> p s f", f=512)
                for s in range(8):
                    nc.vector.bn_stats(out=st[:, s, :], in_=xv[:, s, :])
                mv = stp.tile([128, 2], F32, tag="mv")
                nc.vector.bn_aggr(out=mv, in_=st)
                m2 = stp.tile([128, 2], F32, tag="m2")
                nc.vector.tensor_copy(out=m2[:, 0:1], in_=mv[:, 0:1])
                nc.vector.tensor_mul(out=m2[:, 1:2], in0=mv[:, 0:1], in1=mv[:, 0:1])
                nc.vector.tensor_add(out=m2[:, 1:2], in0=m2[:, 1:2], in1=mv[:, 1:2])
                gsum = stps.tile([8, 2], F32, tag="gsum", bufs=2)
                nc.tensor.matmul(out=gsum, lhsT=gmat_sb, rhs=m2, start=True, stop=True)
                gg = stp.tile([8, 2], F32, tag="gg")   # col0 = S1, col1 = S2 -> r
                nc.vector.tensor_copy(out=gg, in_=gsum)
                t1 = stp.tile([8, 1], F32, tag="t1")
                nc.vector.tensor_mul(out=t1, in0=gg[:, 0:1], in1=gg[:, 0:1])
                t3 = stp.tile([8, 1], F32, tag="t3")
                nc.vector.tensor_scalar(out=t3, in0=gg[:, 1:2], scalar1=float(GS),
                                        scalar2=None, op0=A.mult)
                nc.vector.tensor_sub(out=t3, in0=t3, in1=t1)
                sq = stp.tile([8, 1], F32, tag="sq")
                nc.scalar.activation(out=sq, in_=t3, func=AF.Sqrt, bias=eps_t)
                nc.vector.reciprocal(out=gg[:, 1:2], in_=sq)   # r = rstd/16
                bc = stps.tile([128, 2], F32, tag="bc", bufs=2)
                nc.tensor.matmul(out=bc, lhsT=gmat2_sb, rhs=gg, start=True, stop=True)
                # bc0 = S1_pc = 16*mean_c ; bc1 = r_pc = rstd_c/16
                nc.vector.tensor_scalar(out=a_pc[:, j:j + 1], in0=bc[:, 1:2],
                                        scalar1=gns_pc[:, j:j + 1], scalar2=float(GS),
                                        op0=A.mult, op1=A.mult)
                nc.vector.tensor_mul(out=d_pc[:, j:j + 1], in0=bc[:, 0:1], in1=a_pc[:, j:j + 1])
                nc.vector.tensor_scalar(out=d_pc[:, j:j + 1], in0=d_pc[:, j:j + 1],
                                        scalar1=-1.0 / GS, scalar2=gnb_pc[:, j:j + 1],
                                        op0=A.mult, op1=A.add)
                # row-scale M and W2 by a_c1 in place (gates the projections)
                for wk_ in ("m", "w2"):
                    nc.vector.tensor_scalar(out=w_sb[wk_][:, j, :], in0=w_sb[wk_][:, j, :],
                                            scalar1=a_pc[:, j:j + 1], scalar2=None,
                                            op0=A.mult)
            # d2 = d/a for the output-side bias; d_raw for the v-vector
            nc.vector.reciprocal(out=ra_pc, in_=a_pc)
            nc.vector.tensor_mul(out=ra_pc, in0=ra_pc, in1=d_pc)
            nc.vector.tensor_copy(out=d_bf, in_=ra_pc)            # d/a  (fp16)
            draw_bf = stp.tile([128, CH], BF, tag="draw_bf", bufs=1)
            nc.vector.tensor_copy(out=draw_bf, in_=d_pc)          # raw d (fp16)
            # a as a broadcast tile and as a row (via DRAM bounce)
            scr_a = dram.tile([1, C], F32, name="scr_a", bufs=1)
            nc.scalar.dma_start(out=scr_a[0].rearrange("(a p) -> p a", p=128), in_=a_pc)
            a_row = stp.tile([1, C], F32, tag="a_row", bufs=1)
            nc.scalar.dma_start(out=a_row, in_=scr_a)
            # wv2 = SCALE * a (.) (M^T d + g): the per-key score bias vector
            mv_ps = stps.tile([1, C], F32, tag="mv_ps")
            for j in range(CH):
                nc.tensor.matmul(out=mv_ps, lhsT=draw_bf[:, j:j + 1], rhs=w_sb["mt"][:, j, :],
                                 start=(j == 0), stop=(j == CH - 1))
            wv2_row = stp.tile([1, C], F32, tag="wv2_row", bufs=1)
            nc.vector.tensor_add(out=wv2_row, in0=mv_ps, in1=brow["g"])
            nc.vector.tensor_mul(out=wv2_row, in0=wv2_row, in1=a_row)
            nc.vector.tensor_scalar(out=wv2_row, in0=wv2_row, scalar1=SCALE,
                                    scalar2=None, op0=A.mult)
            scr_w = dram.tile([1, C], F32, name="scr_w", bufs=1)
            nc.scalar.dma_start(out=scr_w, in_=wv2_row)
            wv2_pc = stp.tile([128, CH], F32, tag="wv2_pc", bufs=1)
            nc.scalar.dma_start(out=wv2_pc, in_=scr_w[0].rearrange("(a p) -> p a", p=128))
            wv2_bf = pers.tile([128, CH], BF, name="wv2_bf")
            nc.vector.tensor_copy(out=wv2_bf, in_=wv2_pc)
            # b2 = (d/a) @ W2~ + h2  (per-output-channel bias incl. bo, bv@Wo)
            b2_ps = stps.tile([1, C], F32, tag="mv_ps", name="b2_ps")
            for j in range(CH):
                nc.tensor.matmul(out=b2_ps, lhsT=d_bf[:, j:j + 1], rhs=w_sb["w2"][:, j, :],
                                 start=(j == 0), stop=(j == CH - 1))
            b2_row = stp.tile([1, C], F32, tag="b2_row", bufs=1)
            nc.vector.tensor_add(out=b2_row, in0=b2_ps, in1=brow["h2"])
            scr_b = dram.tile([1, C], F32, name="scr_b", bufs=1)
            nc.scalar.dma_start(out=scr_b, in_=b2_row)
            b2_bc = pers.tile([128, C], F32, name="b2_bc")
            src_b = bass.AP(tensor=scr_b.tensor, offset=scr_b.offset,
                            ap=[[0, 128], [1, C]])
            nc.gpsimd.dma_start(out=b2_bc, in_=src_b)

        # ---------- phase 2: P = M2^T-projection of the queries ----------
        PT = pers.tile([128, CH, NQ], BF, name="PT")
        with tc.tile_pool(name="pps", bufs=6, space="PSUM") as pps:
            for t in range(TT // 2):
                sl = slice(512 * t, 512 * (t + 1))
                for m in range(CH):
                    ps = pps.tile([128, 512], F32, tag="proj", name=f"psp{t}{m}")
                    for j in range(CH):
                        nc.tensor.matmul(out=ps, lhsT=w_sb["m"][:, j, 128 * m:128 * (m + 1)],
                                         rhs=xT[:, j, sl], start=(j == 0), stop=(j == CH - 1))
                    nc.scalar.activation(out=PT[:, m, sl], in_=ps, func=AF.Copy,
                                         scale=a_pc[:, m:m + 1])
            # v = x~^T . wv2 per token: per-key exp bias (emitted after the
            # P-projection so its PSUM use doesn't gate P via bank reuse;
            # v is only needed at the first exp)
            v_row = pers.tile([1, N], F32, name="v_row")
            for t in range(TT):
                v_ps = pps.tile([1, 512], F32, tag="v_ps", bufs=2, name=f"v_ps{t}")
                for j in range(CH):
                    nc.tensor.matmul(out=v_ps, lhsT=wv2_bf[:, j:j + 1],
                                     rhs=xT[:, j, 512 * t:512 * (t + 1)],
                                     start=(j == 0), stop=(j == CH - 1))
                nc.vector.tensor_copy(out=v_row[:, 512 * t:512 * (t + 1)], in_=v_ps)
            scr_v = dram.tile([1, N], F32, name="scr_v", bufs=1)
            nc.scalar.dma_start(out=scr_v, in_=v_row)
            v_pc = pers.tile([128, KC], F32, name="v_pc")
            nc.scalar.dma_start(out=v_pc, in_=scr_v[0].rearrange("(a p) -> p a", p=128))

        # ---------- phase 3: attention + output ----------
        with tc.tile_pool(name="sps", bufs=3, space="PSUM") as sps, \
             tc.tile_pool(name="ops", bufs=1, space="PSUM") as ops, \
             tc.tile_pool(name="attn", bufs=8) as attnp, \
             tc.tile_pool(name="outp", bufs=3) as outp, \
             tc.tile_pool(name="small", bufs=2) as smallp:
            def attn_block(qb0, qw, fast_rse=False):
                qsl = slice(qb0, qb0 + qw)
                ot = [ops.tile([128, 512], F32, tag=f"ot{m}", name=f"ot{m}_{qb0}")
                      for m in range(CH)]
                se_acc = smallp.tile([128, 512], F32, tag="se_acc", bufs=2,
                                     name=f"seacc_{qb0}")
                # software-pipelined: scores/exp for kc+1 are emitted before
                # the attnV consumers of kc so PE never sits on the exp wait.
                at_q = {}
                for kc in range(KC + 1):
                    if kc < KC:
                        sp = sps.tile([128, 512], F32, tag="sp", name=f"sp_{qb0}_{kc}")
                        for j in range(CH):
                            nc.tensor.matmul(out=sp[:, 0:qw], lhsT=xT[:, j, 128 * kc:128 * (kc + 1)],
                                             rhs=PT[:, j, qsl], start=(j == 0), stop=(j == CH - 1))
                        at = attnp.tile([128, 512], BF, tag="at", name=f"at_{qb0}_{kc}")
                        nc.scalar.activation(out=at[:, 0:qw], in_=sp[:, 0:qw],
                                             func=AF.Exp, scale=SCALE,
                                             bias=v_pc[:, kc:kc + 1])
                        if kc == 0:
                            nc.vector.tensor_copy(out=se_acc[:, 0:qw], in_=at[:, 0:qw])
                        else:
                            nc.vector.tensor_add(out=se_acc[:, 0:qw], in0=se_acc[:, 0:qw],
                                                 in1=at[:, 0:qw])
                        at_q[kc] = at
                    if kc >= 1:
                        pc = kc - 1
                        atp = at_q.pop(pc)
                        for m in range(CH):
                            nc.tensor.matmul(out=ot[m][:, 0:qw],
                                             lhsT=xtok_sb[:, pc, 128 * m:128 * (m + 1)],
                                             rhs=atp[:, 0:qw], start=(pc == 0), stop=(pc == KC - 1))
                # 1/sumexp, bounced through DRAM into per-partition layout
                se = sps.tile([128, 512], F32, tag="fp", bufs=1, name=f"se_{qb0}")
                nc.tensor.matmul(out=se[0:1, 0:qw], lhsT=ones_sb, rhs=se_acc[:, 0:qw],
                                 start=True, stop=True)
                rse = smallp.tile([1, 512], F32, tag="rse", name=f"rse_{qb0}")
                nc.vector.reciprocal(out=rse[:, 0:qw], in_=se[0:1, 0:qw])
                rse_pc = smallp.tile([128, 4], F32, tag="rse_pc", name=f"rsepc_{qb0}")
                if fast_rse:
                    # tail-critical: PE-transpose [1,128] slices instead of the
                    # ~3us DRAM round trip (PE is idle at the kernel tail)
                    for s in range(qw // 128):
                        pt = sps.tile([128, 1], F32, tag="sp", name=f"pt_{qb0}_{s}")
                        nc.tensor.transpose(out=pt, in_=rse[0:1, 128 * s:128 * (s + 1)],
                                            identity=ident1)
                        nc.vector.tensor_copy(out=rse_pc[:, s:s + 1], in_=pt)
                else:
                    dscratch = dram.tile([1, 512], F32, tag="dscratch", name=f"dscr_{qb0}")
                    nc.sync.dma_start(out=dscratch[:, 0:qw], in_=rse[:, 0:qw])
                    nc.sync.dma_start(out=rse_pc[:, 0:qw // 128],
                                      in_=dscratch[0, 0:qw].rearrange("(s p) -> p s", p=128))
                osb = outp.tile([128, CH, 512], BF, tag="osb", bufs=2, name=f"osb_{qb0}")
                for m in range(CH):
                    if m % 2 == 0:
                        nc.vector.tensor_copy(out=osb[:, m, 0:qw], in_=ot[m][:, 0:qw])
                    else:
                        nc.scalar.copy(out=osb[:, m, 0:qw], in_=ot[m][:, 0:qw])
                for s in range(qw // 128):
                    q0 = qb0 + 128 * s
                    fp = sps.tile([128, 512], F32, tag="fp", bufs=1, name=f"fp_{q0}")
                    for m in range(CH):
                        nc.tensor.matmul(out=fp, lhsT=osb[:, m, 128 * s:128 * (s + 1)],
                                         rhs=w_sb["w2"][:, m, :], start=(m == 0), stop=(m == CH - 1))
                    res = outp.tile([128, 512], F32, tag="res", name=f"res_{q0}")
                    nc.scalar.dma_start(out=res, in_=xres[q0:q0 + 128, :])
                    nc.gpsimd.tensor_add(out=res, in0=res, in1=b2_bc)
                    fo = outp.tile([128, 512], F32, tag="fo", name=f"fo_{q0}")
                    nc.vector.tensor_scalar(out=fo, in0=fp, scalar1=rse_pc[:, s:s + 1],
                                            scalar2=None, op0=A.mult)
                    nc.vector.tensor_add(out=fo, in0=fo, in1=res)
                    nc.sync.dma_start(out=out[q0:q0 + 128, :], in_=fo)

            # full 512-wide blocks, with the last block split in two 256-wide
            # halves so the first half's output stage overlaps the second
            # half's attention loop (shorter serial tail).
            for qb in range(QB - 1):
                attn_block(512 * qb, 512)
            attn_block(512 * (QB - 1), 256)
            attn_block(512 * (QB - 1) + 256, 256, fast_rse=True)

    nc.compile()
    return nc


def _swizzle_w(w):
    # [C, C] -> [128, CH, C] bf16 lhsT chunks: [ci_local, ci_chunk, co]
    return np.ascontiguousarray(
        np.asarray(w, np.float32).reshape(CH, 128, C).transpose(1, 0, 2)
    ).astype(np.float16 if FP16 else ml_dtypes.bfloat16)


def _chunk_pc(v):
    # [C] -> [128, CH]: column j = channels 128j..128j+127
    return np.ascontiguousarray(np.asarray(v, np.float32).reshape(CH, 128).T)


def _in_maps(x, gn_scale, gn_bias, wq, bq, wk, bk, wv, bv, wo, bo):
    gmat = np.zeros((128, 8), np.float32)
    gmat[np.arange(128), np.arange(128) // GS] = 1.0
    wqf = np.asarray(wq, np.float32); wkf = np.asarray(wk, np.float32)
    wvf = np.asarray(wv, np.float32); wof = np.asarray(wo, np.float32)
    M = wqf @ wkf.T               # S = hn M hn^T
    W2 = wvf @ wof                # (attn hn) W2
    g = wkf @ np.asarray(bq, np.float32)          # per-key bias from bq
    h2 = np.asarray(bv, np.float32) @ wof + np.asarray(bo, np.float32)
    common = {
        "m": _swizzle_w(M), "mt": _swizzle_w(M.T), "w2": _swizzle_w(W2),
        "g": g.reshape(1, C).astype(np.float32),
        "h2": h2.reshape(1, C).astype(np.float32),
        "gns": _chunk_pc(gn_scale), "gnb": _chunk_pc(gn_bias),
        "gmat": gmat, "gmat2": np.ascontiguousarray(gmat.T),
        "ones": np.ones((128, 1), np.float32),
    }
    lp = np.float16 if FP16 else ml_dtypes.bfloat16
    xf = np.asarray(x, np.float32).reshape(B, N, C)
    in_maps = []
    for core in range(8):
        b, h = core // 2, core % 2
        if h == 0:
            xs = xf[b]
        else:
            xs = np.concatenate([xf[b, NQ:], xf[b, :NQ]], axis=0)
        in_maps.append({
            **common,
            "xtb": np.ascontiguousarray(xs.T).astype(lp),
            "xtok": np.ascontiguousarray(xs).astype(lp),
            "xres": np.ascontiguousarray(xs[:NQ]),
        })
    return in_maps


def kernel(x, gn_scale, gn_bias, wq, bq, wk, bk, wv, bv, wo, bo, _trace=False):
    if "nc" not in _CACHE:
        _CACHE["nc"] = _build()
    in_maps = _in_maps(x, gn_scale, gn_bias, wq, bq, wk, bk, wv, bv, wo, bo)
    last_exc = None
    r = None
    for _attempt in range(4):
        if _attempt == 3 and DEDUP_KV:
            # last resort: collective-free variant (pair AllGather is the
            # riskiest feature in a fresh environment)
            if "nc_fallback" not in _CACHE:
                _CACHE["nc_fallback"] = _build(dedup=False)
            nc = _CACHE["nc_fallback"]
        else:
            nc = _CACHE["nc"]
        try:
            r = run_bass_kernel_spmd(nc, in_maps, core_ids=list(range(8)), trace=_trace)
            break
        except Exception as e:  # transient NRT/device hiccups: retry
            last_exc = e
            import time as _time
            _time.sleep(3)
    if r is None:
        raise last_exc
    _CACHE["last_result"] = r
    out = np.empty((B, N, C), np.float32)
    for core in range(8):
        b, h = core // 2, core % 2
        out[b, NQ * h:NQ * (h + 1)] = r.results[core]["out"]
    return out.reshape(B, H, W, C)

